# revision 24
# baseline (speedup 1.0000x reference)
"""Multi-head attention (B=2, T=2048, D=1024, H=16, dh=64) on 8 TRN2 NeuronCores.

Sharding: batch x head-group. Core i handles batch b=i//4 and heads
[4g, 4g+4) with g=i%4. Per core:
  - qk^T = (Wqk_g x^T) in transposed layout [feat, tok], v in [tok, feat]
  - attention with scores kept transposed [k, q]; softmax denominator
    obtained via a ones-column appended to v in the PV matmul
  - exp split between ScalarE (exact) and VectorE (Schraudolph int16
    bitcast to bf16) so neither engine paces the TensorE pipeline
  - heads processed in interleaved pairs; per-pair AllGather of the
    normalized [dh, tok] head outputs across the 4 cores of the batch
    group, overlapping comm with the next pair's compute
  - each core computes its quarter of the rows of the output projection
Host assembles the [2, 2048, 1024] float32 result.
"""
from contextlib import ExitStack

import numpy as np
import ml_dtypes

import concourse.bass as bass
import concourse.mybir as mybir
import concourse.tile as tile
from concourse import bacc
from concourse.bass_utils import run_bass_kernel_spmd

BF16 = mybir.dt.bfloat16
F32 = mybir.dt.float32
I16 = mybir.dt.int16

B, T, D = 2, 2048, 1024
D_HEAD = 64
N_CORES = 8
H_LOC = 4            # heads per core
E_QK = 512           # q+k features per core
E_V = 256            # v features per core
QT = 1024            # q tile (free dim of scores psum)
N_QT = T // QT       # 2
N_KC = T // 128      # 16 k-chunks
N_DC = D // 128      # 8 contraction chunks for projections
T_OUT = T // 4       # 512 rows of output per core

# Schraudolph exp in bf16 bit domain: bf16bits(exp(x)) ~ int16(A*x + B)
EXP_A = float(128.0 / np.log(2.0))
EXP_B = float(127.0 * 128.0 - 5.5)


def build_nc():
    nc = bacc.Bacc("TRN2", target_bir_lowering=False, debug=False,
                   num_devices=N_CORES)

    xt_ext = nc.dram_tensor("xt", [D, T], BF16, kind="ExternalInput")
    wqk_ext = nc.dram_tensor("wqk", [D, E_QK], BF16, kind="ExternalInput")
    wv_ext = nc.dram_tensor("wv", [D, E_V], BF16, kind="ExternalInput")
    wo_ext = nc.dram_tensor("wo", [D, D], BF16, kind="ExternalInput")
    out_ext = nc.dram_tensor("out", [T_OUT, D], F32, kind="ExternalOutput")

    with tile.TileContext(nc) as tc:
        with (
            tc.tile_pool(name="persist", bufs=1) as persist,
            tc.tile_pool(name="work", bufs=4) as work,
            tc.tile_pool(name="dram", bufs=1, space="DRAM") as dram,
        ):
            # ---- load inputs (wo last: only needed at the end) --------------
            xt_sb = persist.tile([128, N_DC, T], BF16)
            wqk_sb = persist.tile([128, N_DC, E_QK], BF16)
            wv_sb = persist.tile([128, N_DC, E_V], BF16)
            wo_sb = persist.tile([128, N_DC, D], BF16)
            for c in range(N_DC):
                nc.sync.dma_start(out=wqk_sb[:, c, :], in_=wqk_ext[128 * c:128 * (c + 1), :])
            for t in range(4):
                for c in range(N_DC):
                    nc.sync.dma_start(out=xt_sb[:, c, 512 * t:512 * (t + 1)],
                                      in_=xt_ext[128 * c:128 * (c + 1), 512 * t:512 * (t + 1)])
            for c in range(N_DC):
                nc.scalar.dma_start(out=wv_sb[:, c, :], in_=wv_ext[128 * c:128 * (c + 1), :])
            for c in range(N_DC):
                nc.gpsimd.dma_start(out=wo_sb[:, c, :], in_=wo_ext[128 * c:128 * (c + 1), :])

            # ---- qk^T projection: [E_QK, T] ---------------------------------
            proj_stack = ExitStack()
            ps_proj = proj_stack.enter_context(
                tc.tile_pool(name="ps_proj", bufs=3, space="PSUM"))
            qkt_sb = persist.tile([128, E_QK // 128, T], BF16)
            for e in range(E_QK // 128):
                for t in range(T // 512):
                    ps = ps_proj.tile([128, 512], F32, tag="ps")
                    for c in range(N_DC):
                        nc.tensor.matmul(
                            ps[:],
                            wqk_sb[:, c, 128 * e:128 * (e + 1)],
                            xt_sb[:, c, 512 * t:512 * (t + 1)],
                            start=(c == 0), stop=(c == N_DC - 1),
                        )
                    nc.vector.tensor_copy(qkt_sb[:, e, 512 * t:512 * (t + 1)], ps[:])

            # zero-padded per-head k^T: head h's features at its own
            # partition range, zeros elsewhere -> K=128 full-array matmuls
            ktz_sb = persist.tile([128, H_LOC, T], BF16)
            nc.vector.memset(ktz_sb[:], 0.0)
            for h in range(H_LOC):
                kt_part = 64 * (h % 2)
                kt_chunk = 2 + h // 2
                nc.vector.tensor_copy(ktz_sb[kt_part:kt_part + 64, h, :],
                                      qkt_sb[kt_part:kt_part + 64, kt_chunk, :])

            # ---- v projection (+ ones column): vext [tok, H_LOC*128] --------
            # per head: [v (64) | ones (1) | padding (63)] -> M=128 PV matmuls
            vext_sb = persist.tile([128, N_KC, H_LOC * 128], BF16)
            nc.vector.memset(vext_sb[:], 1.0)
            for tk in range(N_KC):
                ps_full = ps_proj.tile([128, 512], F32, tag="ps", name="ps_v")
                ps = ps_full[:, 0:E_V]
                for c in range(N_DC):
                    nc.tensor.matmul(
                        ps[:],
                        xt_sb[:, c, 128 * tk:128 * (tk + 1)],
                        wv_sb[:, c, :],
                        start=(c == 0), stop=(c == N_DC - 1),
                    )
                dst = vext_sb[:, tk, :].rearrange("p (h c) -> p h c", h=H_LOC)[:, :, 0:64]

                src = ps[:].rearrange("p (h c) -> p h c", c=64)
                nc.vector.tensor_copy(dst, src)

            proj_stack.close()

            # ---- attention, head pairs interleaved --------------------------
            attn_stack = ExitStack()
            ps_s = attn_stack.enter_context(
                tc.tile_pool(name="ps_s", bufs=2, space="PSUM"))
            ps_o = attn_stack.enter_context(
                tc.tile_pool(name="ps_o", bufs=2, space="PSUM"))
            # block-major bounce layout: 4 blocks of [128 rows, 512 cols]
            # per pair; each core's output quarter lives in exactly one block
            ag_in = [dram.tile([4 * 2 * D_HEAD, T_OUT], BF16, name=f"ag_in_{p}")
                     for p in range(2)]
            pid = nc.partition_id()
            row_b = (pid // 4) * 512          # batch group's rows inside a block
            blk = pid % 4                     # block holding my T quarter
            af_sb = persist.tile([128, N_DC, T_OUT], BF16)
            ag_out = [nc.dram_tensor(f"ag_out_{p}",
                                     [4 * N_CORES * 2 * D_HEAD, T_OUT],
                                     BF16, addr_space="Shared")
                      for p in range(2)]
            for pair in range(2):
                heads = (2 * pair, 2 * pair + 1)
                for qt in range(N_QT):
                    po = {}
                    for h in heads:
                        po[h] = ps_o.tile([128, QT], F32, name=f"po_{h}_{qt}",
                                          tag="po")
                    for kc in range(N_KC):
                        p_tiles = {}
                        s_ps = {}
                        for h in heads:
                            s_ps[h] = ps_s.tile([128, QT], F32,
                                                name=f"s_{h}_{qt}_{kc}", tag="s")
                        for h in heads:
                            for qh in range(QT // 512):
                                qt_chunk = h // 2
                                nc.tensor.matmul(
                                    s_ps[h][:, 512 * qh:512 * (qh + 1)],
                                    ktz_sb[:, h, 128 * kc:128 * (kc + 1)],
                                    qkt_sb[:, qt_chunk,
                                           QT * qt + 512 * qh:QT * qt + 512 * (qh + 1)],
                                    start=True, stop=True,
                                )
                        for h in heads:
                            ps = s_ps[h]
                            p_sb = work.tile([128, QT], BF16, tag="p",
                                             name=f"p_{h}_{qt}_{kc}")
                            if h % 2 == 0:
                                nc.scalar.activation(p_sb[:], ps[:],
                                                     mybir.ActivationFunctionType.Exp)
                            else:
                                nc.vector.tensor_scalar(
                                    p_sb[:].bitcast(I16), ps[:],
                                    EXP_A, EXP_B,
                                    mybir.AluOpType.mult, mybir.AluOpType.add,
                                )
                            p_tiles[h] = p_sb
                        for h in heads:
                            for qh in range(QT // 512):
                                nc.tensor.matmul(
                                    po[h][0:128, 512 * qh:512 * (qh + 1)],
                                    vext_sb[:, kc, 128 * h:128 * (h + 1)],
                                    p_tiles[h][:, 512 * qh:512 * (qh + 1)],
                                    start=(kc == 0), stop=(kc == N_KC - 1),
                                )
                    # normalize: free the PSUM bank fast (copy to SBUF),
                    # then broadcast denominator and multiply by approx recip
                    for h in heads:
                        po_sb = work.tile([65, QT], F32, tag="po_sb",
                                          name=f"posb_{h}_{qt}")
                        nc.scalar.copy(po_sb[:], po[h][0:65, :])
                        den_dram = dram.tile([1, QT], F32, tag="den_dram", bufs=2,
                                             name=f"den_{h}_{qt}")
                        nc.sync.dma_start(out=den_dram[:], in_=po_sb[64:65, :])
                        rb = work.tile([64, QT], F32, tag="rb", name=f"rb_{h}_{qt}")
                        nc.sync.dma_start(out=rb[:],
                                          in_=den_dram[0:1, :].partition_broadcast(64))
                        rc = work.tile([64, QT], F32, tag="rc", name=f"rc_{h}_{qt}")
                        nc.vector.reciprocal_approx_fast(rc[:], rb[:])
                        attn = work.tile([64, QT], BF16, tag="attn",
                                         name=f"attn_{h}_{qt}")
                        nc.vector.tensor_mul(attn[:], po_sb[0:64, :], rc[:])
                        for qb in range(2):
                            b_i = 2 * qt + qb
                            nc.sync.dma_start(
                                out=ag_in[pair][128 * b_i + 64 * (h % 2):
                                                128 * b_i + 64 * (h % 2) + 64, :],
                                in_=attn[:, 512 * qb:512 * (qb + 1)],
                            )
                    for qb in range(2):
                        b_i = 2 * qt + qb
                        nc.gpsimd.collective_compute(
                            "AllGather",
                            mybir.AluOpType.bypass,
                            replica_groups=[list(range(N_CORES))],
                            ins=[ag_in[pair][128 * b_i:128 * (b_i + 1), :].opt()],
                            outs=[ag_out[pair].ap()[1024 * b_i:1024 * (b_i + 1), :].opt()],
                        )
                for c in range(4):
                    nc.gpsimd.dma_start(
                        out=af_sb[:, 2 * c + pair, :],
                        in_=ag_out[pair][bass.ds(blk * 1024 + row_b + 128 * c, 128), :],
                    )
            attn_stack.close()

            # ---- output projection for my T-quarter -------------------------
            final_stack = ExitStack()
            ps_f = final_stack.enter_context(
                tc.tile_pool(name="ps_f", bufs=1, space="PSUM"))
            out_sb = persist.tile([128, 4 * D], F32)
            ps_tiles = {}
            # pass A: pair-0 chunks (even) accumulate while AG_1 is in flight
            for ts in range(T_OUT // 128):
                for n in range(D // 512):
                    ps = ps_f.tile([128, 512], F32, tag=f"ps_{ts}_{n}",
                                   name=f"psf_{ts}_{n}")
                    ps_tiles[(ts, n)] = ps
                    for ci, c in enumerate([0, 2, 4, 6]):
                        nc.tensor.matmul(
                            ps[:],
                            af_sb[:, c, 128 * ts:128 * (ts + 1)],
                            wo_sb[:, c, 512 * n:512 * (n + 1)],
                            start=(ci == 0), stop=False,
                        )
            # pass B: pair-1 chunks (odd) complete each accumulation
            for ts in range(T_OUT // 128):
                for n in range(D // 512):
                    ps = ps_tiles[(ts, n)]
                    for ci, c in enumerate([1, 3, 5, 7]):
                        nc.tensor.matmul(
                            ps[:],
                            af_sb[:, c, 128 * ts:128 * (ts + 1)],
                            wo_sb[:, c, 512 * n:512 * (n + 1)],
                            start=False, stop=(ci == 3),
                        )
                    nc.vector.tensor_copy(
                        out_sb[:, D * ts + 512 * n:D * ts + 512 * (n + 1)], ps[:])
                nc.gpsimd.dma_start(
                    out=out_ext[128 * ts:128 * (ts + 1), :],
                    in_=out_sb[:, D * ts:D * (ts + 1)],
                )
            final_stack.close()

    nc.compile()
    return nc


_NC = None


def _get_nc():
    global _NC
    if _NC is None:
        _NC = build_nc()
    return _NC


def kernel(x, Wqkv, Wo):
    bf16 = ml_dtypes.bfloat16
    s = np.float32(1.0 / np.sqrt(D_HEAD))

    xt = [np.ascontiguousarray(np.asarray(x)[b].T).astype(bf16) for b in range(B)]
    wo = np.ascontiguousarray(np.asarray(Wo).T).astype(bf16)
    Wqkv = np.asarray(Wqkv)

    in_maps = []
    for i in range(N_CORES):
        b, g = divmod(i, 4)
        wq = Wqkv[256 * g:256 * (g + 1)] * s
        wk = Wqkv[D + 256 * g:D + 256 * (g + 1)]
        wqk = np.ascontiguousarray(np.concatenate([wq, wk], axis=0).T).astype(bf16)
        wv = np.ascontiguousarray(Wqkv[2 * D + 256 * g:2 * D + 256 * (g + 1)].T).astype(bf16)
        in_maps.append({"xt": xt[b], "wqk": wqk, "wv": wv, "wo": wo})

    nc = _get_nc()
    res = run_bass_kernel_spmd(nc, in_maps, core_ids=list(range(N_CORES)))

    out = np.empty((B, T, D), dtype=np.float32)
    for i in range(N_CORES):
        b, r = divmod(i, 4)
        out[b, T_OUT * r:T_OUT * (r + 1), :] = res.results[i]["out"]
    return out


# revision 25
# speedup vs baseline: 1.0723x; 1.0723x over previous
"""Multi-head attention (B=2, T=2048, D=1024, H=16, dh=64) on 8 TRN2 NeuronCores.

Sharding: batch x head-group. Core i handles batch b=i//4 and heads
[4g, 4g+4) with g=i%4. Per core:
  - qk^T = (Wqk_g x^T) in transposed layout [feat, tok], v in [tok, feat]
  - attention with scores kept transposed [k, q]; softmax denominator
    obtained via a ones-column appended to v in the PV matmul
  - exp split between ScalarE (exact) and VectorE (Schraudolph int16
    bitcast to bf16) so neither engine paces the TensorE pipeline
  - heads processed in interleaved pairs; per-pair AllGather of the
    normalized [dh, tok] head outputs across the 4 cores of the batch
    group, overlapping comm with the next pair's compute
  - each core computes its quarter of the rows of the output projection
Host assembles the [2, 2048, 1024] float32 result.
"""
from contextlib import ExitStack

import numpy as np
import ml_dtypes

import concourse.bass as bass
import concourse.mybir as mybir
import concourse.tile as tile
from concourse import bacc
from concourse.bass_utils import run_bass_kernel_spmd

BF16 = mybir.dt.bfloat16
F32 = mybir.dt.float32
I16 = mybir.dt.int16

B, T, D = 2, 2048, 1024
D_HEAD = 64
N_CORES = 8
H_LOC = 4            # heads per core
E_QK = 512           # q+k features per core
E_V = 256            # v features per core
QT = 1024            # q tile (free dim of scores psum)
N_QT = T // QT       # 2
N_KC = T // 128      # 16 k-chunks
N_DC = D // 128      # 8 contraction chunks for projections
T_OUT = T // 4       # 512 rows of output per core

# Schraudolph exp in bf16 bit domain: bf16bits(exp(x)) ~ int16(A*x + B)
EXP_A = float(128.0 / np.log(2.0))
EXP_B = float(127.0 * 128.0 - 5.5)


def build_nc():
    nc = bacc.Bacc("TRN2", target_bir_lowering=False, debug=False,
                   num_devices=N_CORES)

    xt_ext = nc.dram_tensor("xt", [D, T], BF16, kind="ExternalInput")
    wqk_ext = nc.dram_tensor("wqk", [D, E_QK], BF16, kind="ExternalInput")
    wv_ext = nc.dram_tensor("wv", [D, E_V], BF16, kind="ExternalInput")
    wo_ext = nc.dram_tensor("wo", [D, D], BF16, kind="ExternalInput")
    out_ext = nc.dram_tensor("out", [T_OUT, D], F32, kind="ExternalOutput")

    with tile.TileContext(nc) as tc:
        with (
            tc.tile_pool(name="persist", bufs=1) as persist,
            tc.tile_pool(name="work", bufs=4) as work,
            tc.tile_pool(name="dram", bufs=1, space="DRAM") as dram,
        ):
            # ---- load inputs (wo last: only needed at the end) --------------
            xt_sb = persist.tile([128, N_DC, T], BF16)
            wqk_sb = persist.tile([128, N_DC, E_QK], BF16)
            wv_sb = persist.tile([128, N_DC, E_V], BF16)
            wo_sb = persist.tile([128, N_DC, D], BF16)
            for c in range(N_DC):
                nc.sync.dma_start(out=xt_sb[:, c, :], in_=xt_ext[128 * c:128 * (c + 1), :])
                nc.sync.dma_start(out=wqk_sb[:, c, :], in_=wqk_ext[128 * c:128 * (c + 1), :])
                nc.sync.dma_start(out=wv_sb[:, c, :], in_=wv_ext[128 * c:128 * (c + 1), :])
            for c in range(N_DC):
                nc.gpsimd.dma_start(out=wo_sb[:, c, :], in_=wo_ext[128 * c:128 * (c + 1), :])

            # ---- qk^T projection: [E_QK, T] ---------------------------------
            proj_stack = ExitStack()
            ps_proj = proj_stack.enter_context(
                tc.tile_pool(name="ps_proj", bufs=3, space="PSUM"))
            qkt_sb = persist.tile([128, E_QK // 128, T], BF16)
            for e in range(E_QK // 128):
                for t in range(T // 512):
                    ps = ps_proj.tile([128, 512], F32, tag="ps")
                    for c in range(N_DC):
                        nc.tensor.matmul(
                            ps[:],
                            wqk_sb[:, c, 128 * e:128 * (e + 1)],
                            xt_sb[:, c, 512 * t:512 * (t + 1)],
                            start=(c == 0), stop=(c == N_DC - 1),
                        )
                    nc.vector.tensor_copy(qkt_sb[:, e, 512 * t:512 * (t + 1)], ps[:])

            # zero-padded per-head k^T: head h's features at its own
            # partition range, zeros elsewhere -> K=128 full-array matmuls
            ktz_sb = persist.tile([128, H_LOC, T], BF16)
            nc.vector.memset(ktz_sb[:], 0.0)
            for h in range(H_LOC):
                kt_part = 64 * (h % 2)
                kt_chunk = 2 + h // 2
                nc.vector.tensor_copy(ktz_sb[kt_part:kt_part + 64, h, :],
                                      qkt_sb[kt_part:kt_part + 64, kt_chunk, :])

            # ---- v projection (+ ones column): vext [tok, H_LOC*128] --------
            # per head: [v (64) | ones (1) | padding (63)] -> M=128 PV matmuls
            vext_sb = persist.tile([128, N_KC, H_LOC * 128], BF16)
            nc.vector.memset(vext_sb[:], 1.0)
            for tk in range(N_KC):
                ps_full = ps_proj.tile([128, 512], F32, tag="ps", name="ps_v")
                ps = ps_full[:, 0:E_V]
                for c in range(N_DC):
                    nc.tensor.matmul(
                        ps[:],
                        xt_sb[:, c, 128 * tk:128 * (tk + 1)],
                        wv_sb[:, c, :],
                        start=(c == 0), stop=(c == N_DC - 1),
                    )
                dst = vext_sb[:, tk, :].rearrange("p (h c) -> p h c", h=H_LOC)[:, :, 0:64]

                src = ps[:].rearrange("p (h c) -> p h c", c=64)
                nc.vector.tensor_copy(dst, src)

            proj_stack.close()

            # ---- attention, head pairs interleaved --------------------------
            attn_stack = ExitStack()
            ps_s = attn_stack.enter_context(
                tc.tile_pool(name="ps_s", bufs=2, space="PSUM"))
            ps_o = attn_stack.enter_context(
                tc.tile_pool(name="ps_o", bufs=2, space="PSUM"))
            # block-major bounce layout: 4 blocks of [128 rows, 512 cols]
            # per pair; each core's output quarter lives in exactly one block
            ag_in = [dram.tile([4 * 2 * D_HEAD, T_OUT], BF16, name=f"ag_in_{p}")
                     for p in range(2)]
            pid = nc.partition_id()
            row_b = (pid // 4) * 512          # batch group's rows inside a block
            blk = pid % 4                     # block holding my T quarter
            af_sb = persist.tile([128, N_DC, T_OUT], BF16)
            ag_out = [nc.dram_tensor(f"ag_out_{p}",
                                     [4 * N_CORES * 2 * D_HEAD, T_OUT],
                                     BF16, addr_space="Shared")
                      for p in range(2)]
            for pair in range(2):
                heads = (2 * pair, 2 * pair + 1)
                for qt in range(N_QT):
                    po = {}
                    for h in heads:
                        po[h] = ps_o.tile([128, QT], F32, name=f"po_{h}_{qt}",
                                          tag="po")
                    for kc in range(N_KC):
                        p_tiles = {}
                        s_ps = {}
                        for h in heads:
                            s_ps[h] = ps_s.tile([128, QT], F32,
                                                name=f"s_{h}_{qt}_{kc}", tag="s")
                        for h in heads:
                            for qh in range(QT // 512):
                                qt_chunk = h // 2
                                nc.tensor.matmul(
                                    s_ps[h][:, 512 * qh:512 * (qh + 1)],
                                    ktz_sb[:, h, 128 * kc:128 * (kc + 1)],
                                    qkt_sb[:, qt_chunk,
                                           QT * qt + 512 * qh:QT * qt + 512 * (qh + 1)],
                                    start=True, stop=True,
                                )
                        for h in heads:
                            ps = s_ps[h]
                            p_sb = work.tile([128, QT], BF16, tag="p",
                                             name=f"p_{h}_{qt}_{kc}")
                            if h % 2 == 0:
                                nc.scalar.activation(p_sb[:], ps[:],
                                                     mybir.ActivationFunctionType.Exp)
                            else:
                                nc.vector.tensor_scalar(
                                    p_sb[:].bitcast(I16), ps[:],
                                    EXP_A, EXP_B,
                                    mybir.AluOpType.mult, mybir.AluOpType.add,
                                )
                            p_tiles[h] = p_sb
                        for h in heads:
                            for qh in range(QT // 512):
                                nc.tensor.matmul(
                                    po[h][0:128, 512 * qh:512 * (qh + 1)],
                                    vext_sb[:, kc, 128 * h:128 * (h + 1)],
                                    p_tiles[h][:, 512 * qh:512 * (qh + 1)],
                                    start=(kc == 0), stop=(kc == N_KC - 1),
                                )
                    # normalize: free the PSUM bank fast (copy to SBUF),
                    # then broadcast denominator and multiply by approx recip
                    for h in heads:
                        po_sb = work.tile([65, QT], F32, tag="po_sb",
                                          name=f"posb_{h}_{qt}")
                        nc.scalar.copy(po_sb[:], po[h][0:65, :])
                        den_dram = dram.tile([1, QT], F32, tag="den_dram", bufs=2,
                                             name=f"den_{h}_{qt}")
                        nc.sync.dma_start(out=den_dram[:], in_=po_sb[64:65, :])
                        rb = work.tile([64, QT], F32, tag="rb", name=f"rb_{h}_{qt}")
                        nc.sync.dma_start(out=rb[:],
                                          in_=den_dram[0:1, :].partition_broadcast(64))
                        rc = work.tile([64, QT], F32, tag="rc", name=f"rc_{h}_{qt}")
                        nc.vector.reciprocal_approx_fast(rc[:], rb[:])
                        attn = work.tile([64, QT], BF16, tag="attn",
                                         name=f"attn_{h}_{qt}")
                        nc.vector.tensor_mul(attn[:], po_sb[0:64, :], rc[:])
                        for qb in range(2):
                            b_i = 2 * qt + qb
                            nc.sync.dma_start(
                                out=ag_in[pair][128 * b_i + 64 * (h % 2):
                                                128 * b_i + 64 * (h % 2) + 64, :],
                                in_=attn[:, 512 * qb:512 * (qb + 1)],
                            )
                    for qb in range(2):
                        b_i = 2 * qt + qb
                        nc.gpsimd.collective_compute(
                            "AllGather",
                            mybir.AluOpType.bypass,
                            replica_groups=[list(range(N_CORES))],
                            ins=[ag_in[pair][128 * b_i:128 * (b_i + 1), :].opt()],
                            outs=[ag_out[pair].ap()[1024 * b_i:1024 * (b_i + 1), :].opt()],
                        )
                for c in range(4):
                    nc.gpsimd.dma_start(
                        out=af_sb[:, 2 * c + pair, :],
                        in_=ag_out[pair][bass.ds(blk * 1024 + row_b + 128 * c, 128), :],
                    )
            attn_stack.close()

            # ---- output projection for my T-quarter -------------------------
            final_stack = ExitStack()
            ps_f = final_stack.enter_context(
                tc.tile_pool(name="ps_f", bufs=1, space="PSUM"))
            out_sb = persist.tile([128, 4 * D], F32)
            ps_tiles = {}
            # pass A: pair-0 chunks (even) accumulate while AG_1 is in flight
            for ts in range(T_OUT // 128):
                for n in range(D // 512):
                    ps = ps_f.tile([128, 512], F32, tag=f"ps_{ts}_{n}",
                                   name=f"psf_{ts}_{n}")
                    ps_tiles[(ts, n)] = ps
                    for ci, c in enumerate([0, 2, 4, 6]):
                        nc.tensor.matmul(
                            ps[:],
                            af_sb[:, c, 128 * ts:128 * (ts + 1)],
                            wo_sb[:, c, 512 * n:512 * (n + 1)],
                            start=(ci == 0), stop=False,
                        )
            # pass B: pair-1 chunks (odd) complete each accumulation
            for ts in range(T_OUT // 128):
                for n in range(D // 512):
                    ps = ps_tiles[(ts, n)]
                    for ci, c in enumerate([1, 3, 5, 7]):
                        nc.tensor.matmul(
                            ps[:],
                            af_sb[:, c, 128 * ts:128 * (ts + 1)],
                            wo_sb[:, c, 512 * n:512 * (n + 1)],
                            start=False, stop=(ci == 3),
                        )
                    nc.vector.tensor_copy(
                        out_sb[:, D * ts + 512 * n:D * ts + 512 * (n + 1)], ps[:])
                nc.gpsimd.dma_start(
                    out=out_ext[128 * ts:128 * (ts + 1), :],
                    in_=out_sb[:, D * ts:D * (ts + 1)],
                )
            final_stack.close()

    nc.compile()
    return nc


_NC = None


def _get_nc():
    global _NC
    if _NC is None:
        _NC = build_nc()
    return _NC


def kernel(x, Wqkv, Wo):
    bf16 = ml_dtypes.bfloat16
    s = np.float32(1.0 / np.sqrt(D_HEAD))

    xt = [np.ascontiguousarray(np.asarray(x)[b].T).astype(bf16) for b in range(B)]
    wo = np.ascontiguousarray(np.asarray(Wo).T).astype(bf16)
    Wqkv = np.asarray(Wqkv)

    in_maps = []
    for i in range(N_CORES):
        b, g = divmod(i, 4)
        wq = Wqkv[256 * g:256 * (g + 1)] * s
        wk = Wqkv[D + 256 * g:D + 256 * (g + 1)]
        wqk = np.ascontiguousarray(np.concatenate([wq, wk], axis=0).T).astype(bf16)
        wv = np.ascontiguousarray(Wqkv[2 * D + 256 * g:2 * D + 256 * (g + 1)].T).astype(bf16)
        in_maps.append({"xt": xt[b], "wqk": wqk, "wv": wv, "wo": wo})

    nc = _get_nc()
    res = run_bass_kernel_spmd(nc, in_maps, core_ids=list(range(N_CORES)))

    out = np.empty((B, T, D), dtype=np.float32)
    for i in range(N_CORES):
        b, r = divmod(i, 4)
        out[b, T_OUT * r:T_OUT * (r + 1), :] = res.results[i]["out"]
    return out


# revision 27
# speedup vs baseline: 1.0888x; 1.0154x over previous
"""Multi-head attention (B=2, T=2048, D=1024, H=16, dh=64) on 8 TRN2 NeuronCores.

Sharding: batch x head-group. Core i handles batch b=i//4 and heads
[4g, 4g+4) with g=i%4. Per core:
  - qk^T = (Wqk_g x^T) in transposed layout [feat, tok], v in [tok, feat]
  - attention with scores kept transposed [k, q]; softmax denominator
    obtained via a ones-column appended to v in the PV matmul
  - exp split between ScalarE (exact) and VectorE (Schraudolph int16
    bitcast to bf16) so neither engine paces the TensorE pipeline
  - heads processed in interleaved pairs; per-pair AllGather of the
    normalized [dh, tok] head outputs across the 4 cores of the batch
    group, overlapping comm with the next pair's compute
  - each core computes its quarter of the rows of the output projection
Host assembles the [2, 2048, 1024] float32 result.
"""
from contextlib import ExitStack

import numpy as np
import ml_dtypes

import concourse.bass as bass
import concourse.mybir as mybir
import concourse.tile as tile
from concourse import bacc
from concourse.bass_utils import run_bass_kernel_spmd

BF16 = mybir.dt.bfloat16
F32 = mybir.dt.float32
I16 = mybir.dt.int16

B, T, D = 2, 2048, 1024
D_HEAD = 64
N_CORES = 8
H_LOC = 4            # heads per core
E_QK = 512           # q+k features per core
E_V = 256            # v features per core
QT = 1024            # q tile (free dim of scores psum)
N_QT = T // QT       # 2
N_KC = T // 128      # 16 k-chunks
N_DC = D // 128      # 8 contraction chunks for projections
T_OUT = T // 4       # 512 rows of output per core

# Schraudolph exp in bf16 bit domain: bf16bits(exp(x)) ~ int16(A*x + B)
EXP_A = float(128.0 / np.log(2.0))
EXP_B = float(127.0 * 128.0 - 5.5)


def build_nc():
    nc = bacc.Bacc("TRN2", target_bir_lowering=False, debug=False,
                   num_devices=N_CORES)

    xt_ext = nc.dram_tensor("xt", [D, T], BF16, kind="ExternalInput")
    wqk_ext = nc.dram_tensor("wqk", [D, E_QK], BF16, kind="ExternalInput")
    wv_ext = nc.dram_tensor("wv", [D, E_V], BF16, kind="ExternalInput")
    wo_ext = nc.dram_tensor("wo", [D, D], BF16, kind="ExternalInput")
    out_ext = nc.dram_tensor("out", [T_OUT, D], F32, kind="ExternalOutput")

    with tile.TileContext(nc) as tc:
        with (
            tc.tile_pool(name="persist", bufs=1) as persist,
            tc.tile_pool(name="work", bufs=4) as work,
            tc.tile_pool(name="dram", bufs=1, space="DRAM") as dram,
        ):
            # ---- load inputs (wo last: only needed at the end) --------------
            xt_sb = persist.tile([128, N_DC, T], BF16)
            wqk_sb = persist.tile([128, N_DC, E_QK], BF16)
            wv_sb = persist.tile([128, N_DC, E_V], BF16)
            wo_sb = persist.tile([128, N_DC, D], BF16)
            for c in range(N_DC):
                nc.sync.dma_start(out=xt_sb[:, c, :], in_=xt_ext[128 * c:128 * (c + 1), :])
                nc.sync.dma_start(out=wqk_sb[:, c, :], in_=wqk_ext[128 * c:128 * (c + 1), :])
                nc.sync.dma_start(out=wv_sb[:, c, :], in_=wv_ext[128 * c:128 * (c + 1), :])
            for c in range(N_DC):
                nc.gpsimd.dma_start(out=wo_sb[:, c, :], in_=wo_ext[128 * c:128 * (c + 1), :])

            # ---- qk^T projection: [E_QK, T] ---------------------------------
            proj_stack = ExitStack()
            ps_proj = proj_stack.enter_context(
                tc.tile_pool(name="ps_proj", bufs=3, space="PSUM"))
            # PE warmup: dependency-free matmuls on whatever is in SBUF keep
            # the HAM activity monitor at full clock while input DMAs land
            warm_ps = ps_proj.tile([128, 512], F32, tag="ps", name="warm_ps")
            for w in range(16):
                nc.tensor.matmul(warm_ps[:], xt_sb[:, 0, 0:128],
                                 xt_sb[:, 1, 0:512], start=(w == 0),
                                 stop=(w == 15), skip_group_check=True)
            qkt_sb = persist.tile([128, E_QK // 128, T], BF16)
            for e in range(E_QK // 128):
                for t in range(T // 512):
                    ps = ps_proj.tile([128, 512], F32, tag="ps")
                    for c in range(N_DC):
                        nc.tensor.matmul(
                            ps[:],
                            wqk_sb[:, c, 128 * e:128 * (e + 1)],
                            xt_sb[:, c, 512 * t:512 * (t + 1)],
                            start=(c == 0), stop=(c == N_DC - 1),
                        )
                    nc.vector.tensor_copy(qkt_sb[:, e, 512 * t:512 * (t + 1)], ps[:])

            # zero-padded per-head k^T: head h's features at its own
            # partition range, zeros elsewhere -> K=128 full-array matmuls
            ktz_sb = persist.tile([128, H_LOC, T], BF16)
            nc.vector.memset(ktz_sb[:], 0.0)
            for h in range(H_LOC):
                kt_part = 64 * (h % 2)
                kt_chunk = 2 + h // 2
                nc.vector.tensor_copy(ktz_sb[kt_part:kt_part + 64, h, :],
                                      qkt_sb[kt_part:kt_part + 64, kt_chunk, :])

            # ---- v projection (+ ones column): vext [tok, H_LOC*128] --------
            # per head: [v (64) | ones (1) | padding (63)] -> M=128 PV matmuls
            vext_sb = persist.tile([128, N_KC, H_LOC * 128], BF16)
            nc.vector.memset(vext_sb[:], 1.0)
            for tk in range(N_KC):
                ps_full = ps_proj.tile([128, 512], F32, tag="ps", name="ps_v")
                ps = ps_full[:, 0:E_V]
                for c in range(N_DC):
                    nc.tensor.matmul(
                        ps[:],
                        xt_sb[:, c, 128 * tk:128 * (tk + 1)],
                        wv_sb[:, c, :],
                        start=(c == 0), stop=(c == N_DC - 1),
                    )
                dst = vext_sb[:, tk, :].rearrange("p (h c) -> p h c", h=H_LOC)[:, :, 0:64]

                src = ps[:].rearrange("p (h c) -> p h c", c=64)
                nc.vector.tensor_copy(dst, src)

            proj_stack.close()

            # ---- attention, head pairs interleaved --------------------------
            attn_stack = ExitStack()
            ps_s = attn_stack.enter_context(
                tc.tile_pool(name="ps_s", bufs=2, space="PSUM"))
            ps_o = attn_stack.enter_context(
                tc.tile_pool(name="ps_o", bufs=2, space="PSUM"))
            # block-major bounce layout: 4 blocks of [128 rows, 512 cols]
            # per pair; each core's output quarter lives in exactly one block
            ag_in = [dram.tile([4 * 2 * D_HEAD, T_OUT], BF16, name=f"ag_in_{p}")
                     for p in range(2)]
            pid = nc.partition_id()
            row_b = (pid // 4) * 512          # batch group's rows inside a block
            blk = pid % 4                     # block holding my T quarter
            af_sb = persist.tile([128, N_DC, T_OUT], BF16)
            ag_out = [nc.dram_tensor(f"ag_out_{p}",
                                     [4 * N_CORES * 2 * D_HEAD, T_OUT],
                                     BF16, addr_space="Shared")
                      for p in range(2)]
            for pair in range(2):
                heads = (2 * pair, 2 * pair + 1)
                for qt in range(N_QT):
                    po = {}
                    for h in heads:
                        po[h] = ps_o.tile([128, QT], F32, name=f"po_{h}_{qt}",
                                          tag="po")
                    prev_p = None
                    for kc in range(N_KC):
                        p_tiles = {}
                        s_ps = {}
                        for h in heads:
                            s_ps[h] = ps_s.tile([128, QT], F32,
                                                name=f"s_{h}_{qt}_{kc}", tag="s")
                        for h in heads:
                            for qh in range(QT // 512):
                                qt_chunk = h // 2
                                nc.tensor.matmul(
                                    s_ps[h][:, 512 * qh:512 * (qh + 1)],
                                    ktz_sb[:, h, 128 * kc:128 * (kc + 1)],
                                    qkt_sb[:, qt_chunk,
                                           QT * qt + 512 * qh:QT * qt + 512 * (qh + 1)],
                                    start=True, stop=True,
                                )
                        for h in heads:
                            ps = s_ps[h]
                            p_sb = work.tile([128, QT], BF16, tag="p", bufs=6,
                                             name=f"p_{h}_{qt}_{kc}")
                            if h % 2 == 0:
                                nc.scalar.activation(p_sb[:], ps[:],
                                                     mybir.ActivationFunctionType.Exp)
                            else:
                                nc.vector.tensor_scalar(
                                    p_sb[:].bitcast(I16), ps[:],
                                    EXP_A, EXP_B,
                                    mybir.AluOpType.mult, mybir.AluOpType.add,
                                )
                            p_tiles[h] = p_sb
                        if prev_p is not None:
                            for h in heads:
                                for qh in range(QT // 512):
                                    nc.tensor.matmul(
                                        po[h][0:128, 512 * qh:512 * (qh + 1)],
                                        vext_sb[:, kc - 1, 128 * h:128 * (h + 1)],
                                        prev_p[h][:, 512 * qh:512 * (qh + 1)],
                                        start=(kc == 1), stop=False,
                                    )
                        prev_p = p_tiles
                    for h in heads:
                        for qh in range(QT // 512):
                            nc.tensor.matmul(
                                po[h][0:128, 512 * qh:512 * (qh + 1)],
                                vext_sb[:, N_KC - 1, 128 * h:128 * (h + 1)],
                                prev_p[h][:, 512 * qh:512 * (qh + 1)],
                                start=False, stop=True,
                            )
                    # normalize: free the PSUM bank fast (copy to SBUF),
                    # then broadcast denominator and multiply by approx recip
                    for h in heads:
                        po_sb = work.tile([65, QT], F32, tag="po_sb",
                                          name=f"posb_{h}_{qt}")
                        nc.scalar.copy(po_sb[:], po[h][0:65, :])
                        den_dram = dram.tile([1, QT], F32, tag="den_dram", bufs=2,
                                             name=f"den_{h}_{qt}")
                        nc.sync.dma_start(out=den_dram[:], in_=po_sb[64:65, :])
                        rb = work.tile([64, QT], F32, tag="rb", name=f"rb_{h}_{qt}")
                        nc.sync.dma_start(out=rb[:],
                                          in_=den_dram[0:1, :].partition_broadcast(64))
                        rc = work.tile([64, QT], F32, tag="rc", name=f"rc_{h}_{qt}")
                        nc.vector.reciprocal_approx_fast(rc[:], rb[:])
                        attn = work.tile([64, QT], BF16, tag="attn",
                                         name=f"attn_{h}_{qt}")
                        nc.vector.tensor_mul(attn[:], po_sb[0:64, :], rc[:])
                        for qb in range(2):
                            b_i = 2 * qt + qb
                            nc.sync.dma_start(
                                out=ag_in[pair][128 * b_i + 64 * (h % 2):
                                                128 * b_i + 64 * (h % 2) + 64, :],
                                in_=attn[:, 512 * qb:512 * (qb + 1)],
                            )
                    for qb in range(2):
                        b_i = 2 * qt + qb
                        nc.gpsimd.collective_compute(
                            "AllGather",
                            mybir.AluOpType.bypass,
                            replica_groups=[list(range(N_CORES))],
                            ins=[ag_in[pair][128 * b_i:128 * (b_i + 1), :].opt()],
                            outs=[ag_out[pair].ap()[1024 * b_i:1024 * (b_i + 1), :].opt()],
                        )
                for c in range(4):
                    nc.gpsimd.dma_start(
                        out=af_sb[:, 2 * c + pair, :],
                        in_=ag_out[pair][bass.ds(blk * 1024 + row_b + 128 * c, 128), :],
                    )
            attn_stack.close()

            # ---- output projection for my T-quarter -------------------------
            final_stack = ExitStack()
            ps_f = final_stack.enter_context(
                tc.tile_pool(name="ps_f", bufs=1, space="PSUM"))
            out_sb = persist.tile([128, 4 * D], F32)
            ps_tiles = {}
            # pass A: pair-0 chunks (even) accumulate while AG_1 is in flight
            for ts in range(T_OUT // 128):
                for n in range(D // 512):
                    ps = ps_f.tile([128, 512], F32, tag=f"ps_{ts}_{n}",
                                   name=f"psf_{ts}_{n}")
                    ps_tiles[(ts, n)] = ps
                    for ci, c in enumerate([0, 2, 4, 6]):
                        nc.tensor.matmul(
                            ps[:],
                            af_sb[:, c, 128 * ts:128 * (ts + 1)],
                            wo_sb[:, c, 512 * n:512 * (n + 1)],
                            start=(ci == 0), stop=False,
                        )
            # pass B: pair-1 chunks (odd) complete each accumulation
            for ts in range(T_OUT // 128):
                for n in range(D // 512):
                    ps = ps_tiles[(ts, n)]
                    for ci, c in enumerate([1, 3, 5, 7]):
                        nc.tensor.matmul(
                            ps[:],
                            af_sb[:, c, 128 * ts:128 * (ts + 1)],
                            wo_sb[:, c, 512 * n:512 * (n + 1)],
                            start=False, stop=(ci == 3),
                        )
                    nc.vector.tensor_copy(
                        out_sb[:, D * ts + 512 * n:D * ts + 512 * (n + 1)], ps[:])
                nc.gpsimd.dma_start(
                    out=out_ext[128 * ts:128 * (ts + 1), :],
                    in_=out_sb[:, D * ts:D * (ts + 1)],
                )
            final_stack.close()

    nc.compile()
    return nc


_NC = None


def _get_nc():
    global _NC
    if _NC is None:
        _NC = build_nc()
    return _NC


def kernel(x, Wqkv, Wo):
    bf16 = ml_dtypes.bfloat16
    s = np.float32(1.0 / np.sqrt(D_HEAD))

    xt = [np.ascontiguousarray(np.asarray(x)[b].T).astype(bf16) for b in range(B)]
    wo = np.ascontiguousarray(np.asarray(Wo).T).astype(bf16)
    Wqkv = np.asarray(Wqkv)

    in_maps = []
    for i in range(N_CORES):
        b, g = divmod(i, 4)
        wq = Wqkv[256 * g:256 * (g + 1)] * s
        wk = Wqkv[D + 256 * g:D + 256 * (g + 1)]
        wqk = np.ascontiguousarray(np.concatenate([wq, wk], axis=0).T).astype(bf16)
        wv = np.ascontiguousarray(Wqkv[2 * D + 256 * g:2 * D + 256 * (g + 1)].T).astype(bf16)
        in_maps.append({"xt": xt[b], "wqk": wqk, "wv": wv, "wo": wo})

    nc = _get_nc()
    res = run_bass_kernel_spmd(nc, in_maps, core_ids=list(range(N_CORES)))

    out = np.empty((B, T, D), dtype=np.float32)
    for i in range(N_CORES):
        b, r = divmod(i, 4)
        out[b, T_OUT * r:T_OUT * (r + 1), :] = res.results[i]["out"]
    return out
